# revision 7
# baseline (speedup 1.0000x reference)
"""Causal multi-head attention block (B=4, S=2048, D=1024, H=16) on 8 TRN2
cores, optimized for the axon relay's per-launch cost model: measured launch
time = fixed floor + ~0.08ms/MB of (input+output wire bytes) + ~0.4ms per
buffer; on-device execution (~500us) is invisible below the relay's
completion-poll quantum. So the design minimizes wire bytes and buffer count
(2 buffers/core, ~40MB total wire vs 130MB for naive full-I/O):

 - fp16 everywhere (wire + intermediates): absmax-rel ~5e-4 (beats bf16 7x).
 - ONE input tensor per core (~3.03MB): its half of one batch's xT
   (features split across the pair), ONE 1MB weight piece (wq|wk|wv|wp for
   its head group, by core-pair index), the 128x128 causal mask, packed
   biases. Unique data only: x (16MB) and weights (8MB) are sharded across
   cores and reconstructed on device with AllGathers (pair-gather for x,
   4-way gather for weight pieces) -- on-device interconnect is free.
 - ONE output tensor per core (2MB): the pair's partial outputs are summed
   on device with a pair ReduceScatter, each core ships half the rows of
   the final [2048,1024] f16 output. Host just concatenates + adds b_proj.
Total wire: ~41MB/launch vs 130MB for the naive full-I/O layout.

Sharding: data-parallel over batch (4) x tensor-parallel over head groups
(2). Core c: batch b=c//2, heads (c%2)*8..+8. Compute pipeline per core is
unchanged from the bf16 baseline (feature-major, zero on-chip transposes,
fp32 PSUM): qkv proj -> per-head scores with diagonal-band masking ->
exp on ACT -> PV with ones-column denominator trick -> normalize ->
output projection; PE-dense phases interleaved into the ACT-bound phase2.
"""

import numpy as np
import ml_dtypes

import concourse.bass as bass
import concourse.tile as tile
from concourse import bacc, mybir

F32 = mybir.dt.float32
F16 = mybir.dt.float16

B, S, D = 4, 2048, 1024
H = 16
HD = D // H           # 64
HPC = 8               # heads per core
DC = HPC * HD         # 512 per-core head dims
NB = S // 512         # 4 query/key 512-blocks
NT = S // 128         # 16 seq 128-tiles
KO = D // 128         # 8 contraction tiles for qkv proj
SCALE = 1.0 / np.sqrt(HD)

# shard layout (f16 elements)
X_ELEMS = 512 * S             # 1048576: xT rows hg*512..+512 of this batch
W_ELEMS = 1024 * 512          # 524288: one weight piece
M_ELEMS = 128 * 128           # 16384: causal mask for the diagonal tile
B_ELEMS = 3 * DC              # 1536: bq, bk, bv for this head group
X_OFF = 0
W_OFF = X_ELEMS
M_OFF = W_OFF + W_ELEMS
B_OFF = M_OFF + M_ELEMS
SHARD_ELEMS = B_OFF + B_ELEMS

_CACHE = {}


def _build():
    nc = bacc.Bacc("TRN2", target_bir_lowering=False, debug=False, num_devices=8,
                   enable_partition_id=False)

    shard = nc.dram_tensor("shard", [SHARD_ELEMS], F16, kind="ExternalInput")
    out = nc.dram_tensor("out", [S // 2, D], F16, kind="ExternalOutput")

    sh = shard.ap()

    with tile.TileContext(nc) as tc:
        with tc.tile_pool(name="dram", bufs=1, space="DRAM") as dram, \
             tc.tile_pool(name="persist", bufs=1) as persist, \
             tc.tile_pool(name="xk_pool", bufs=16) as xk_pool, \
             tc.tile_pool(name="e_pool", bufs=8) as e_pool, \
             tc.tile_pool(name="r_pool", bufs=6) as r_pool, \
             tc.tile_pool(name="o_pool", bufs=4) as o_pool, \
             tc.tile_pool(name="ps_acc", bufs=3, space="PSUM") as ps_acc, \
             tc.tile_pool(name="ps_sc", bufs=2, space="PSUM") as ps_sc, \
             tc.tile_pool(name="ps_av", bufs=1, space="PSUM") as ps_av:

            # ---- on-device input reconstruction via collectives ----
            xb = dram.tile([512, S], F16)          # my xT half (bounce)
            wb = dram.tile([W_ELEMS], F16)         # my weight piece (bounce)
            xg = dram.tile([2, 512, S], F16)       # gathered full xT
            wg = dram.tile([4, W_ELEMS], F16)      # gathered wq,wk,wv,wp
            nc.gpsimd.dma_start(
                xb[:], sh[X_OFF:X_OFF + X_ELEMS].rearrange("(p s) -> p s", p=512))
            nc.gpsimd.dma_start(
                wb[:], sh[W_OFF:W_OFF + W_ELEMS])
            nc.gpsimd.collective_compute(
                "AllGather", mybir.AluOpType.bypass,
                replica_groups=[[0, 1], [2, 3], [4, 5], [6, 7]],
                ins=[xb[:].opt()], outs=[xg[:].opt()],
            )
            nc.gpsimd.collective_compute(
                "AllGather", mybir.AluOpType.bypass,
                replica_groups=[[0, 2, 4, 6], [1, 3, 5, 7]],
                ins=[wb[:].opt()], outs=[wg[:].opt()],
            )

            # weight piece r: [1024 rows, 512 cols] row-major, rows = (ko p)
            wq_r = wg[0].rearrange("(ko p m) -> p ko m", p=128, m=512)
            wk_r = wg[1].rearrange("(ko p m) -> p ko m", p=128, m=512)
            wv_r = wg[2].rearrange("(ko p m) -> p ko m", p=128, m=512)
            # wp piece: [512 rows, 1024 cols] row-major
            wp_r = wg[3].rearrange("(ko p n) -> p ko n", p=128, n=1024)

            def xg_tile(ko, s0, s1):
                # xT row (ko*128+p) -> gathered region ko//4, row (ko%4)*128+p
                return xg[ko // 4, (ko % 4) * 128:(ko % 4 + 1) * 128, s0:s1]

            # ---- persistent SBUF ----
            wq_sb = persist.tile([128, KO, DC], F16)
            wk_sb = persist.tile([128, KO, DC], F16)
            wv_sb = persist.tile([128, KO, DC], F16)
            bq_sb = persist.tile([128, DC // 128], F32)
            bk_sb = persist.tile([128, DC // 128], F32)
            bvb_sb = persist.tile([128, DC], F32)
            qT_sb = persist.tile([128, DC // 128, S], F16)
            kT_sb = persist.tile([128, DC // 128, S], F16)
            v_sb = persist.tile([128, NT, HPC, 65], F16)
            avT_sb = persist.tile([128, DC // 128, S], F16)
            wp_sb = persist.tile([128, DC // 128, D], F16)
            mask_sb = persist.tile([128, 128], F16)

            # partial output + pair-reduced half in DRAM
            part = dram.tile([S, D], F16)
            red = dram.tile([S // 2, D], F16)

            # DMA emission in k-interleaved order so phase-1(0)'s k-outer
            # streaming overlaps its own loads
            xk0 = []
            for k in range(KO):
                t = xk_pool.tile([128, 512], F16, tag="xk")
                nc.sync.dma_start(t[:], xg_tile(k, 0, 512))
                xk0.append(t)
                nc.sync.dma_start(wq_sb[:, k, :], wq_r[:, k, :])
                nc.sync.dma_start(wk_sb[:, k, :], wk_r[:, k, :])
            for k in range(KO):
                nc.sync.dma_start(wv_sb[:, k, :], wv_r[:, k, :])
            # biases: f16 on wire. bq/bk sections are packed [128, 4]
            # (element (p, m) = bias of feature m*128+p, matching the
            # per-partition scalar layout tensor_scalar_add wants); bv is in
            # natural feature order and broadcast-DMA'd across partitions
            # with an f16->f32 cast (only gpsimd DMAs can cast).
            bq16 = persist.tile([128, DC // 128], F16)
            bk16 = persist.tile([128, DC // 128], F16)
            nc.sync.dma_start(
                bq16[:], sh[B_OFF:B_OFF + DC].rearrange("(p m) -> p m", p=128))
            nc.sync.dma_start(
                bk16[:], sh[B_OFF + DC:B_OFF + 2 * DC].rearrange("(p m) -> p m", p=128))
            nc.vector.tensor_copy(bq_sb[:], bq16[:])
            nc.vector.tensor_copy(bk_sb[:], bk16[:])
            nc.gpsimd.dma_start(
                bvb_sb[:],
                bass.AP(tensor=shard, offset=B_OFF + 2 * DC,
                        ap=[[0, 128], [1, DC]]),
            )
            # ones column for the PV denominator trick
            nc.vector.memset(v_sb[:, :, :, 64:65], 1.0)

            # warm the PE (HAM clock ramp) with throwaway matmuls while the
            # first DMAs are in flight
            warm_sb = persist.tile([128, 512], F16)
            nc.vector.memset(warm_sb[:], 0.0)
            for wi in range(2):
                wacc = ps_acc.tile([128, 512], F32, tag="acc", name=f"warm{wi}")
                for _ in range(8):
                    nc.tensor.matmul(wacc[:], warm_sb[:, 0:128], warm_sb[:],
                                     start=True, stop=True)

            _sc_stash = []

            def p1_psum(i):
                r = i % 8
                if r < 2:
                    return ps_acc.tile([128, 512], F32, tag="acc", name=f"p1acc{i}")
                if r < 6:
                    if (r - 2) % 2 == 0:
                        _sc_stash.append(ps_sc.tile([128, 2, 512], F32, tag="sc", name=f"p1sc{i}"))
                    return _sc_stash[-1][:, (r - 2) % 2, :]
                return ps_av.tile([128, 512], F32, tag="av", name=f"p1av{i}")

            def p1_qk_unit(n, xk, which, m, acc=None):
                w_sb, b_sb, dst = ((wq_sb, bq_sb, qT_sb), (wk_sb, bk_sb, kT_sb))[which]
                if acc is None:
                    acc = ps_acc.tile([128, 512], F32, tag="acc")
                for k in range(KO):
                    nc.tensor.matmul(
                        acc[:],
                        w_sb[:, k, m * 128:(m + 1) * 128],
                        xk[k][:],
                        start=(k == 0), stop=(k == KO - 1),
                    )
                nc.vector.tensor_scalar_add(
                    dst[:, m, n * 512:(n + 1) * 512], acc[:], b_sb[:, m:m + 1],
                )

            def p1_v_unit(n, xk, u, acc=None):
                st = n * 4 + u
                if acc is None:
                    acc = ps_acc.tile([128, 512], F32, tag="acc")
                for k in range(KO):
                    nc.tensor.matmul(
                        acc[:],
                        xk[k][:, u * 128:(u + 1) * 128],
                        wv_sb[:, k, :],
                        start=(k == 0), stop=(k == KO - 1),
                    )
                nc.vector.tensor_add(
                    v_sb[:, st, :, 0:64],
                    acc[:].rearrange("p (h d) -> p h d", h=HPC),
                    bvb_sb[:].rearrange("p (h d) -> p h d", h=HPC),
                )

            def phase1_units(n):
                if n == 0:
                    xk = xk0
                else:
                    xk = []
                    for k in range(KO):
                        t = xk_pool.tile([128, 512], F16, tag="xk")
                        nc.sync.dma_start(t[:], xg_tile(k, n * 512, (n + 1) * 512))
                        xk.append(t)

                def mk(i, fn):
                    if n == 0:
                        return lambda: fn(p1_psum(i))
                    return lambda: fn(None)

                i = 0
                for m in range(DC // 128):
                    yield mk(i, lambda acc, m=m: p1_qk_unit(n, xk, 0, m, acc))
                    i += 1
                    yield mk(i, lambda acc, m=m: p1_qk_unit(n, xk, 1, m, acc))
                    i += 1
                for u in range(4):
                    yield mk(i, lambda acc, u=u: p1_v_unit(n, xk, u, acc))
                    i += 1

            def phase2(j, h, tail=False):
                n_full = 4 * j
                pb = (h % 2) * 64
                ko_h = h // 2
                q_rhs = qT_sb[pb:pb + 64, ko_h, j * 512:(j + 1) * 512]
                av = ps_av.tile([66, 512], F32, tag="av")
                first_pv = [True]

                def pv(dst_ap, v_tt, e_ap, last=False):
                    nc.tensor.matmul(
                        dst_ap, v_sb[:, v_tt, h, 0:65], e_ap,
                        start=first_pv[0], stop=last,
                    )
                    first_pv[0] = False

                for cp in range(2):
                    dsc = ps_sc.tile([128, 2, 512], F32, tag="sc")
                    ed = e_pool.tile([128, 2, 512], F16, tag="e")
                    wmax = (cp * 2 + 2) * 128
                    for ci in range(2):
                        c = cp * 2 + ci
                        qc = qT_sb[pb:pb + 64, ko_h,
                                   j * 512 + c * 128:j * 512 + (c + 1) * 128]
                        for dk in range(c + 1):
                            tt = 4 * j + dk
                            nc.tensor.matmul(
                                dsc[:, ci, dk * 128:(dk + 1) * 128],
                                kT_sb[pb:pb + 64, ko_h, tt * 128:(tt + 1) * 128],
                                qc,
                                start=True, stop=True,
                            )
                    nc.scalar.activation(
                        ed[:, :, 0:wmax], dsc[:, :, 0:wmax],
                        mybir.ActivationFunctionType.Exp, scale=float(SCALE),
                    )
                    for ci in range(2):
                        c = cp * 2 + ci
                        nc.vector.tensor_mul(
                            ed[:, ci, c * 128:(c + 1) * 128],
                            ed[:, ci, c * 128:(c + 1) * 128],
                            mask_sb[:, :],
                        )
                        for dk in range(c + 1):
                            pv(av[0:65, c * 128:(c + 1) * 128], 4 * j + dk,
                               ed[:, ci, dk * 128:(dk + 1) * 128],
                               last=(n_full == 0 and c == 3 and dk == 3))
                for tg in range(n_full // 2):
                    sc = ps_sc.tile([128, 2, 512], F32, tag="sc")
                    for u in range(2):
                        tt = tg * 2 + u
                        nc.tensor.matmul(
                            sc[:, u, :],
                            kT_sb[pb:pb + 64, ko_h, tt * 128:(tt + 1) * 128],
                            q_rhs,
                            start=True, stop=True,
                        )
                    e = e_pool.tile([128, 2, 512], F16, tag="e")
                    nc.scalar.activation(
                        e[:], sc[:], mybir.ActivationFunctionType.Exp,
                        scale=float(SCALE),
                    )
                    for u in range(2):
                        tt = tg * 2 + u
                        pv(av[0:65, :], tt, e[:, u, :],
                           last=(n_full and tt == n_full - 1))
                if tail:
                    avc = av
                else:
                    avc = r_pool.tile([65, 512], F32, tag="avc")
                    nc.vector.tensor_copy(avc[:], av[0:65, :])
                rs = r_pool.tile([1, 512], F32, tag="rs")
                nc.vector.reciprocal(rs[:], avc[64:65, :])
                rb = r_pool.tile([64, 512], F32, tag="rb")
                nc.gpsimd.partition_broadcast(rb[:], rs[:])
                nc.vector.tensor_mul(
                    avT_sb[pb:pb + 64, ko_h, j * 512:(j + 1) * 512],
                    avc[0:64, :], rb[:],
                )

            def p3_unit(st, n2, acc=None, tail=False):
                if acc is None:
                    acc = ps_acc.tile([128, 512], F32, tag="acc")
                for k in range(DC // 128):
                    nc.tensor.matmul(
                        acc[:],
                        avT_sb[:, k, st * 128:(st + 1) * 128],
                        wp_sb[:, k, n2 * 512:(n2 + 1) * 512],
                        start=(k == 0), stop=(k == DC // 128 - 1),
                    )
                o = o_pool.tile([128, 512], F16, tag="o")
                if tail:
                    nc.scalar.copy(o[:], acc[:])
                else:
                    nc.any.tensor_copy(out=o[:], in_=acc[:])
                nc.sync.dma_start(
                    part[st * 128:(st + 1) * 128, n2 * 512:(n2 + 1) * 512],
                    o[:],
                )

            def phase3_units(j, steal_psum=False):
                i = 0
                for u in range(4):
                    for n2 in range(D // 512):
                        if steal_psum:
                            yield (lambda st=4 * j + u, n2=n2, i=i:
                                   p3_unit(st, n2, p1_psum(i)))
                        else:
                            yield lambda st=4 * j + u, n2=n2: p3_unit(st, n2)
                        i += 1

            for unit in phase1_units(0):
                unit()
            for k in range(DC // 128):
                nc.sync.dma_start(wp_sb[:, k, :], wp_r[:, k, :])
            nc.sync.dma_start(
                mask_sb[:], sh[M_OFF:M_OFF + M_ELEMS].rearrange("(p f) -> p f", p=128))

            for j in range(NB):
                fillers = []
                if j + 1 < NB:
                    fillers.extend(phase1_units(j + 1))
                else:
                    for jj in range(NB - 1):
                        fillers.extend(phase3_units(jj))
                nf = len(fillers)
                if j + 1 < NB:
                    per_head = [(nf * (h + 1)) // 8 - (nf * h) // 8
                                for h in range(HPC)]
                else:
                    per_head = [0, 0, 2, 3, 4, 5, 5, 5][:HPC]
                    while sum(per_head) < nf:
                        per_head[-1] += 1
                    while sum(per_head) > nf:
                        for i in range(HPC):
                            if per_head[i] > 0 and sum(per_head) > nf:
                                per_head[i] -= 1
                fi = 0
                for h in range(HPC):
                    phase2(j, h)
                    for _ in range(per_head[h]):
                        if fi < nf:
                            fillers[fi]()
                            fi += 1
                while fi < nf:
                    fillers[fi]()
                    fi += 1
            for u in range(4):
                for n2 in range(D // 512):
                    p3_unit(4 * (NB - 1) + u, n2, tail=True)

            # pair-sum the partials on device; each core keeps half the rows
            nc.gpsimd.collective_compute(
                "ReduceScatter", mybir.AluOpType.add,
                replica_groups=[[0, 1], [2, 3], [4, 5], [6, 7]],
                ins=[part[:].opt()], outs=[red[:].opt()],
            )
            for i in range(S // 2 // 128):
                t = o_pool.tile([128, D], F16, tag="of")
                nc.sync.dma_start(t[:], red[i * 128:(i + 1) * 128, :])
                nc.sync.dma_start(out.ap()[i * 128:(i + 1) * 128, :], t[:])

    nc.compile()
    return nc


def _get_nc():
    if "nc" not in _CACHE:
        _CACHE["nc"] = _build()
    return _CACHE["nc"]


class _Runner:
    """One jitted 8-core executable, cached for the process lifetime.
    (Building a second executable for a collective NEFF in one process
    desyncs the axon mesh, so kernel() must reuse a single one.)"""

    def __init__(self, nc):
        import jax
        from jax.sharding import Mesh, PartitionSpec
        from jax.experimental.shard_map import shard_map
        from concourse.bass2jax import (
            _bass_exec_p, install_neuronx_cc_hook, partition_id_tensor)

        install_neuronx_cc_hook()
        self.jax = jax
        partition_name = (nc.partition_id_tensor.name
                          if nc.partition_id_tensor else None)
        in_names, out_names, out_avals, zero_outs = [], [], [], []
        for alloc in nc.m.functions[0].allocations:
            if not isinstance(alloc, mybir.MemoryLocationSet):
                continue
            name = alloc.memorylocations[0].name
            if alloc.kind == "ExternalInput":
                if name != partition_name:
                    in_names.append(name)
            elif alloc.kind == "ExternalOutput":
                shape, dtype = tuple(alloc.tensor_shape), mybir.dt.np(alloc.dtype)
                out_names.append(name)
                out_avals.append(jax.core.ShapedArray(shape, dtype))
                zero_outs.append(np.zeros(shape, dtype))
        self.in_names, self.out_names = in_names, out_names
        self.out_avals = out_avals
        self.concat_zeros = [np.zeros((8 * z.shape[0], *z.shape[1:]), z.dtype)
                             for z in zero_outs]
        n_params, n_outs = len(in_names), len(out_names)
        all_in_names = (in_names + out_names
                        + ([partition_name] if partition_name else []))

        def _body(*args):
            ops = list(args)
            if partition_name:
                ops.append(partition_id_tensor())
            return tuple(_bass_exec_p.bind(
                *ops, out_avals=tuple(out_avals), in_names=tuple(all_in_names),
                out_names=tuple(out_names), lowering_input_output_aliases=(),
                sim_require_finite=True, sim_require_nnan=True, nc=nc))

        mesh = Mesh(np.asarray(jax.devices()[:8]), ("core",))
        self.sharded = jax.jit(
            shard_map(_body, mesh=mesh,
                      in_specs=(PartitionSpec("core"),) * (n_params + n_outs),
                      out_specs=(PartitionSpec("core"),) * n_outs,
                      check_rep=False),
            donate_argnums=tuple(range(n_params, n_params + n_outs)),
            keep_unused=True)

    def run(self, in_maps):
        jax = self.jax
        dev_in = [
            jax.device_put(np.concatenate(
                [np.asarray(in_maps[c][nm]) for c in range(8)], axis=0))
            for nm in self.in_names]
        zs = [jax.device_put(z) for z in self.concat_zeros]
        jax.block_until_ready(zs)
        outs = self.sharded(*dev_in, *zs)
        jax.block_until_ready(outs)
        return [
            {nm: np.asarray(outs[i]).reshape(8, *self.out_avals[i].shape)[c]
             for i, nm in enumerate(self.out_names)}
            for c in range(8)]


def _get_runner():
    if "runner" not in _CACHE:
        _CACHE["runner"] = _Runner(_get_nc())
    return _CACHE["runner"]


def _make_mask():
    tt = np.arange(128)[:, None]
    q = np.arange(128)[None, :]
    return (tt <= q).astype(np.float16)


def make_in_maps(x, w_attn, b_attn, w_proj):
    f16 = np.float16
    mask = _make_mask().reshape(-1)
    wqs = w_attn[:, 0 * D:1 * D]
    wks = w_attn[:, 1 * D:2 * D]
    wvs = w_attn[:, 2 * D:3 * D]
    in_maps = []
    for c in range(8):
        b, hg = c // 2, c % 2
        cs = slice(hg * DC, (hg + 1) * DC)
        xh = np.ascontiguousarray(x[b].T[hg * 512:(hg + 1) * 512, :]).astype(f16)
        wpiece = [wqs[:, cs], wks[:, cs], wvs[:, cs],
                  w_proj[cs, :]][c // 2]
        wpiece = np.ascontiguousarray(wpiece).astype(f16)
        bias = np.concatenate([
            b_attn[0 * D:1 * D][cs].reshape(DC // 128, 128).T.reshape(-1),
            b_attn[1 * D:2 * D][cs].reshape(DC // 128, 128).T.reshape(-1),
            b_attn[2 * D:3 * D][cs],          # natural order for broadcast
        ]).astype(f16)
        shard = np.concatenate(
            [xh.reshape(-1), wpiece.reshape(-1), mask, bias])
        assert shard.shape[0] == SHARD_ELEMS
        in_maps.append({"shard": shard})
    return in_maps


def kernel(x, w_attn, b_attn, w_proj, b_proj):
    x = np.asarray(x, dtype=np.float32)
    w_attn = np.asarray(w_attn, dtype=np.float32)
    b_attn = np.asarray(b_attn, dtype=np.float32)
    w_proj = np.asarray(w_proj, dtype=np.float32)
    b_proj = np.asarray(b_proj, dtype=np.float32)

    import time as _time

    in_maps = make_in_maps(x, w_attn, b_attn, w_proj)

    # Transient relay wedges ("mesh desynced", NRT_EXEC_UNIT_UNRECOVERABLE)
    # happen when a fresh process races the previous one's comm teardown;
    # they clear after a cooldown. Level-2 fallback rebuilds the executable.
    results = None
    for attempt in range(4):
        try:
            results = _get_runner().run(in_maps)
            break
        except Exception:
            if attempt == 3:
                raise
            _time.sleep(10 * (attempt + 1))
            if attempt >= 1:
                _CACHE.pop("runner", None)
    return assemble(results, b_proj)


def assemble(results, b_proj):
    b_proj = np.asarray(b_proj, dtype=np.float32)
    out = np.empty((B, S, D), dtype=np.float32)
    for b in range(B):
        out[b, 0:S // 2] = results[2 * b]["out"].astype(np.float32)
        out[b, S // 2:] = results[2 * b + 1]["out"].astype(np.float32)
        out[b] += b_proj
    return out
